# revision 9
# baseline (speedup 1.0000x reference)
"""EFLSTM Trainium2 kernel: 8-core tensor-parallel LSTM + fused head.

Strategy (8 NeuronCores, SPMD single program, per-core data differs):
  - Tensor-parallel over the 4H gate dimension: core k owns gate rows
    [k*128:(k+1)*128] of each of the four gates (i, f, g, o), so it computes
    h rows [k*128:(k+1)*128] each timestep.  A per-step AllGather rebuilds
    the full transposed hidden state h_T [H=1024, B=128] on every core.
  - Input projection x @ W_ih.T is folded into the recurrent PSUM
    accumulation (x is pre-transposed on the host with a ones-row so the
    bias is folded into the matmul as well).  The projection matmuls do not
    depend on h, so they execute during the AllGather latency window.
  - Matmul operands and the h exchange are bf16 (fp32 PSUM accumulation,
    fp32 cell state); gate nonlinearities are split per gate chunk so the
    c-update overlaps the o-gate matmuls.
  - FC head is fused per-step: fc1 (one 128-wide slice of C per core; cores
    k and k+4 duplicate a slice, fc2 weights pre-scaled by 0.5 so the final
    ReduceScatter sum is correct) + fc2 partials accumulate into a
    token-major buffer; after the loop one ReduceScatter + log_softmax on
    each core's token shard produces the output.
"""

import numpy as np
import ml_dtypes

import concourse.bacc as bacc
import concourse.mybir as mybir
import concourse.tile as tile
from concourse.bass_utils import run_bass_kernel_spmd

F32 = mybir.dt.float32
BF16 = mybir.dt.bfloat16
AF = mybir.ActivationFunctionType
ALU = mybir.AluOpType

N_CORES = 8
B = 128
T = 512
DIMS = (300, 74, 35)
D = sum(DIMS)  # 409
DP = D + 1     # 410: extra ones-row folds the gate bias into the matmul
H = 1024
G = 4 * H
C = 512
O = 7
HSL = H // N_CORES        # 128 h rows per core
GSL = 4 * HSL             # 512 gate rows per core
KC_X = [128, 128, 128, DP - 3 * 128]   # contraction chunks over D+1
NKX = len(KC_X)
NKH = H // 128            # 8 contraction chunks over H
TOK = B * T


import os
NO_CC = bool(int(os.environ.get("EF_NO_CC", "0")))


def build_kernel(t_steps=T):
    nc = bacc.Bacc("TRN2", target_bir_lowering=False, debug=False,
                   num_devices=N_CORES)

    xT = nc.dram_tensor("xT", [t_steps, 128, NKX, B], BF16, kind="ExternalInput")
    wihT = nc.dram_tensor("wihT", [NKX, 128, GSL], BF16, kind="ExternalInput")
    whhT = nc.dram_tensor("whhT", [NKH, 128, GSL], BF16, kind="ExternalInput")
    fc1wT = nc.dram_tensor("fc1wT", [NKH, 128, 128], BF16, kind="ExternalInput")
    fc1b = nc.dram_tensor("fc1b", [128, 1], F32, kind="ExternalInput")
    fc2wT = nc.dram_tensor("fc2wT", [128, O], BF16, kind="ExternalInput")
    fc2b = nc.dram_tensor("fc2b", [128, O], F32, kind="ExternalInput")

    n_tok_loc = B * t_steps // N_CORES
    out_sh = nc.dram_tensor("out_sh", [n_tok_loc, O], F32, kind="ExternalOutput")

    with tile.TileContext(nc) as tc:
        with (
            tc.tile_pool(name="const", bufs=1) as const,
            tc.tile_pool(name="xtp", bufs=3) as xtp,
            tc.tile_pool(name="pg", bufs=2, space="PSUM") as pgp,
            tc.tile_pool(name="ph", bufs=2, space="PSUM") as php,
            tc.tile_pool(name="pf", bufs=2, space="PSUM") as pfp,
            tc.tile_pool(name="ew", bufs=2) as ewp,
            tc.tile_pool(name="htp", bufs=3) as htp,
            tc.tile_pool(name="dram", bufs=2, space="DRAM") as dramp,
            tc.tile_pool(name="dram1", bufs=1, space="DRAM") as dramp1,
        ):
            # ---- resident weights ----
            wih_sb = const.tile([128, NKX, GSL], BF16)
            nc.sync.dma_start(wih_sb[:], wihT[:].rearrange("k p g -> p k g"))
            whh_sb = const.tile([128, NKH, GSL], BF16)
            nc.sync.dma_start(whh_sb[:], whhT[:].rearrange("k p g -> p k g"))
            fc1_sb = const.tile([128, NKH, 128], BF16)
            nc.sync.dma_start(fc1_sb[:], fc1wT[:].rearrange("k p c -> p k c"))
            fc1b_sb = const.tile([128, 1], F32)
            nc.sync.dma_start(fc1b_sb[:], fc1b[:])
            fc2w_sb = const.tile([128, O], BF16)
            nc.sync.dma_start(fc2w_sb[:], fc2wT[:])
            fc2b_sb = const.tile([128, O], F32)
            nc.sync.dma_start(fc2b_sb[:], fc2b[:])

            c_sb = const.tile([128, 128], F32)   # persistent cell state slice
            # head partials accumulate in SBUF; one bulk store after the loop
            out_acc = const.tile([128, t_steps, O], F32)

            out_part = dramp1.tile([n_tok_loc * N_CORES, O], F32)

            hT_prev = None
            gate_slices = [(mc * 128, (mc + 1) * 128) for mc in range(4)]

            for t in range(t_steps):
                # -- projection matmuls for step t (independent of h) --
                xts = xtp.tile([128, NKX, B], BF16)
                nc.sync.dma_start(xts[:], xT[t])
                pg = pgp.tile([128, GSL], F32)
                for mc in range(4):
                    lo, hi = gate_slices[mc]
                    for kc in range(NKX):
                        kk = KC_X[kc]
                        nc.tensor.matmul(
                            pg[:, lo:hi],
                            wih_sb[:kk, kc, lo:hi],
                            xts[:kk, kc, :],
                            start=(kc == 0),
                            stop=(t == 0 and kc == NKX - 1),
                        )
                # -- recurrent matmuls (consume previous gathered h) --
                if t > 0:
                    for mc in range(4):
                        lo, hi = gate_slices[mc]
                        for kc in range(NKH):
                            nc.tensor.matmul(
                                pg[:, lo:hi],
                                whh_sb[:, kc, lo:hi],
                                hT_prev[:, kc, :],
                                start=False,
                                stop=(kc == NKH - 1),
                            )

                # -- head for step t-1 (emitted after rec so PE prioritizes rec) --
                if t > 0:
                    emit_head(nc, php, pfp, ewp, fc1_sb, fc1b_sb, fc2w_sb,
                              hT_prev, out_acc, t - 1)

                # -- elementwise: gates -> h slice (i,f,g,o gate chunk order;
                #    c-update runs while the o-gate matmuls finish) --
                gnl = ewp.tile([128, GSL], F32)
                nc.scalar.activation(gnl[:, 0:256], pg[:, 0:256], AF.Sigmoid)
                if t > 0:
                    fcs = ewp.tile([128, 128], F32)
                    nc.vector.tensor_mul(fcs[:], gnl[:, 128:256], c_sb[:])
                nc.scalar.activation(gnl[:, 256:384], pg[:, 256:384], AF.Tanh)
                ig = ewp.tile([128, 128], F32)
                nc.vector.tensor_mul(ig[:], gnl[:, 0:128], gnl[:, 256:384])
                if t == 0:
                    nc.vector.tensor_copy(c_sb[:], ig[:])
                else:
                    nc.vector.tensor_add(c_sb[:], fcs[:], ig[:])
                tc_t = ewp.tile([128, 128], F32)
                nc.scalar.activation(tc_t[:], c_sb[:], AF.Tanh)
                nc.scalar.activation(gnl[:, 384:512], pg[:, 384:512], AF.Sigmoid)
                h_sl = ewp.tile([128, 128], BF16)
                nc.vector.tensor_mul(h_sl[:], gnl[:, 384:512], tc_t[:])

                # -- exchange: AllGather h_T across the 8 cores --
                bnc_in = dramp.tile([128, B], BF16)
                nc.sync.dma_start(bnc_in[:], h_sl[:])
                bnc_out = dramp.tile(
                    [H, B], BF16, addr_space="Local" if NO_CC else "Shared")
                if NO_CC:
                    for kc in range(NKH):
                        nc.gpsimd.dma_start(
                            bnc_out[kc * 128:(kc + 1) * 128, :], bnc_in[:])
                else:
                    nc.gpsimd.collective_compute(
                        "AllGather",
                        ALU.bypass,
                        replica_groups=[list(range(N_CORES))],
                        ins=[bnc_in[:].opt()],
                        outs=[bnc_out[:].opt()],
                    )
                hT = htp.tile([128, NKH, B], BF16)
                nc.sync.dma_start(
                    hT[:], bnc_out[:].rearrange("(k p) b -> p k b", p=128))
                hT_prev = hT

            # head for the final step
            emit_head(nc, php, pfp, ewp, fc1_sb, fc1b_sb, fc2w_sb,
                      hT_prev, out_acc, t_steps - 1)

            # bulk store of the accumulated head partials (token-major)
            nc.sync.dma_start(
                out_part[:].rearrange("(b t) o -> b t o", t=t_steps),
                out_acc[:])

            # ---- tail: ReduceScatter fc2 partials, bias + log_softmax ----
            rs_out = dramp1.tile([n_tok_loc, O], F32)
            nc.gpsimd.collective_compute(
                "ReduceScatter",
                ALU.add,
                replica_groups=[list(range(N_CORES))],
                ins=[out_part[:].opt()],
                outs=[rs_out[:].opt()],
            )
            # token rows viewed as [128, CH, O]: partition p owns tokens
            # p*CH..(p+1)*CH-1 — contiguous per-partition DMA lines
            CH = n_tok_loc // 128
            z_sb = ewp.tile([128, CH, O], F32)
            nc.sync.dma_start(
                z_sb[:], rs_out[:].rearrange("(p c) o -> p c o", p=128))
            res_sb = ewp.tile([128, CH, O], F32)
            for ch in range(CH):
                zb = ewp.tile([128, O], F32)
                nc.vector.tensor_add(zb[:], z_sb[:, ch, :], fc2b_sb[:])
                mx = ewp.tile([128, 1], F32)
                nc.vector.reduce_max(mx[:], zb[:], axis=mybir.AxisListType.X)
                sh = ewp.tile([128, O], F32)
                nc.vector.tensor_scalar_sub(sh[:], zb[:], mx[:])
                ex = ewp.tile([128, O], F32)
                nc.scalar.activation(ex[:], sh[:], AF.Exp)
                sm = ewp.tile([128, 1], F32)
                nc.vector.reduce_sum(sm[:], ex[:], axis=mybir.AxisListType.X)
                lg = ewp.tile([128, 1], F32)
                nc.scalar.activation(lg[:], sm[:], AF.Ln)
                nc.vector.tensor_scalar_sub(res_sb[:, ch, :], sh[:], lg[:])
            nc.sync.dma_start(
                out_sh[:].rearrange("(p c) o -> p c o", p=128), res_sb[:])

    nc.compile()
    return nc


def emit_head(nc, php, pfp, ewp, fc1_sb, fc1b_sb, fc2w_sb, hT, out_acc, t):
    """fc1 (C slice) + relu + fc2 partial for timestep t → SBUF accumulator."""
    ph = php.tile([128, B], F32)
    for kc in range(NKH):
        nc.tensor.matmul(ph[:], fc1_sb[:, kc, :], hT[:, kc, :],
                         start=(kc == 0), stop=(kc == NKH - 1))
    hid = ewp.tile([128, B], BF16)
    nc.scalar.activation(hid[:], ph[:], AF.Relu, bias=fc1b_sb[:])
    pf = pfp.tile([128, O], F32)
    nc.tensor.matmul(pf[:], hid[:], fc2w_sb[:], start=True, stop=True)
    nc.vector.tensor_copy(out_acc[:, t, :], pf[:])


_CACHED = {}


def _get_kernel(t_steps):
    if t_steps not in _CACHED:
        _CACHED[t_steps] = build_kernel(t_steps)
    return _CACHED[t_steps]


def prep_inputs(m_text, m_audio, m_video, W_ih, W_hh, b_ih, b_hh,
                fc1_w, fc1_b, fc2_w, fc2_b, t_steps=T):
    """Host-side layout prep; returns per-core input maps."""
    bf = ml_dtypes.bfloat16
    x = np.concatenate([np.asarray(m_text), np.asarray(m_audio),
                        np.asarray(m_video)], axis=-1).astype(np.float32)
    b_, t_, d_ = x.shape
    assert (b_, d_) == (B, D) and t_ == t_steps
    # x_T: [T, D+1(ones) padded to 4*128, B], stored [t, p, k, b] so the
    # per-step SBUF load is one contiguous 1KB line per partition
    xTf = np.zeros((t_steps, NKX * 128, B), np.float32)
    xTf[:, :D, :] = x.transpose(1, 2, 0)
    xTf[:, D, :] = 1.0
    xTf = np.ascontiguousarray(
        xTf.reshape(t_steps, NKX, 128, B).transpose(0, 2, 1, 3)).astype(bf)

    W_ih = np.asarray(W_ih, np.float32)
    W_hh = np.asarray(W_hh, np.float32)
    bias = (np.asarray(b_ih) + np.asarray(b_hh)).astype(np.float32)
    fc1_w = np.asarray(fc1_w, np.float32)
    fc1_b = np.asarray(fc1_b, np.float32)
    fc2_w = np.asarray(fc2_w, np.float32)
    fc2_b = np.asarray(fc2_b, np.float32)

    in_maps = []
    gate_order = (0, 1, 2, 3)  # i, f, g, o (pytorch row-block order)
    for k in range(N_CORES):
        rows = np.concatenate(
            [np.arange(g * H + k * HSL, g * H + (k + 1) * HSL)
             for g in gate_order])
        wih_sl = W_ih[rows, :]            # [512, 409]
        whh_sl = W_hh[rows, :]            # [512, 1024]
        b_sl = bias[rows]                 # [512]
        wihT_k = np.zeros((NKX * 128, GSL), np.float32)
        wihT_k[:D, :] = wih_sl.T
        wihT_k[D, :] = b_sl
        wihT_k = np.ascontiguousarray(
            wihT_k.reshape(NKX, 128, GSL)).astype(bf)
        whhT_k = np.ascontiguousarray(
            whh_sl.T.reshape(NKH, 128, GSL)).astype(bf)

        cc = k % 4                        # C chunk (cores k and k+4 duplicate)
        crows = np.arange(cc * 128, (cc + 1) * 128)
        fc1wT_k = np.ascontiguousarray(
            fc1_w[crows, :].T.reshape(NKH, 128, 128)).astype(bf)
        fc1b_k = np.ascontiguousarray(fc1_b[crows].reshape(128, 1))
        fc2wT_k = np.ascontiguousarray(0.5 * fc2_w[:, crows].T).astype(bf)
        fc2b_k = np.ascontiguousarray(
            np.broadcast_to(fc2_b[None, :], (128, O))).astype(np.float32)

        in_maps.append({
            "xT": xTf,
            "wihT": wihT_k,
            "whhT": whhT_k,
            "fc1wT": fc1wT_k,
            "fc1b": fc1b_k,
            "fc2wT": fc2wT_k,
            "fc2b": fc2b_k,
        })
    return in_maps


def run(inputs, t_steps=T, trace=False):
    nc = _get_kernel(t_steps)
    in_maps = prep_inputs(
        inputs["m_text"], inputs["m_audio"], inputs["m_video"],
        inputs["W_ih"], inputs["W_hh"], inputs["b_ih"], inputs["b_hh"],
        inputs["fc1_w"], inputs["fc1_b"], inputs["fc2_w"], inputs["fc2_b"],
        t_steps=t_steps)
    res = run_bass_kernel_spmd(
        nc, in_maps, core_ids=list(range(N_CORES)), trace=trace)
    shards = [res.results[k]["out_sh"] for k in range(N_CORES)]
    full = np.concatenate(shards, axis=0)          # [(b t), O] token-major
    out = full.reshape(B, t_steps, O)
    return out, res


def kernel(**inputs) -> np.ndarray:
    t_steps = np.asarray(inputs["m_text"]).shape[1]
    out, _ = run(inputs, t_steps=t_steps)
    return out.astype(np.float32)



# revision 18
# speedup vs baseline: 1.4104x; 1.4104x over previous
"""EFLSTM Trainium2 kernel: 8-core tensor-parallel LSTM + fused head.

Strategy (8 NeuronCores, SPMD single program, per-core data differs):
  - Tensor-parallel over the 4H gate dimension: core k owns gate rows
    [k*128:(k+1)*128] of each of the four gates (i, f, g, o), so it computes
    h rows [k*128:(k+1)*128] each timestep.  A per-step AllGather rebuilds
    the full transposed hidden state h_T [H=1024, B=128] on every core.
  - Input projection x @ W_ih.T is folded into the recurrent PSUM
    accumulation (x is pre-transposed on the host with a ones-row so the
    bias is folded into the matmul as well).  The projection matmuls do not
    depend on h, so they execute during the AllGather latency window.
  - Matmul operands and the h exchange are bf16 (fp32 PSUM accumulation,
    fp32 cell state); gate nonlinearities are split per gate chunk so the
    c-update overlaps the o-gate matmuls.
  - FC head is fused per-step: fc1 (one 128-wide slice of C per core; cores
    k and k+4 duplicate a slice, fc2 weights pre-scaled by 0.5 so the final
    ReduceScatter sum is correct) + fc2 partials accumulate into a
    token-major buffer; after the loop one ReduceScatter + log_softmax on
    each core's token shard produces the output.
"""

import numpy as np
import ml_dtypes

import concourse.bacc as bacc
import concourse.mybir as mybir
import concourse.tile as tile
from concourse.bass_utils import run_bass_kernel_spmd

F32 = mybir.dt.float32
BF16 = mybir.dt.bfloat16
AF = mybir.ActivationFunctionType
ALU = mybir.AluOpType

N_CORES = 8
B = 128
T = 512
DIMS = (300, 74, 35)
D = sum(DIMS)  # 409
DP = D + 1     # 410: extra ones-row folds the gate bias into the matmul
H = 1024
G = 4 * H
C = 512
O = 7
HSL = H // N_CORES        # 128 h rows per core
GSL = 4 * HSL             # 512 gate rows per core
KC_X = [128, 128, 128, DP - 3 * 128]   # contraction chunks over D+1
NKX = len(KC_X)
NKH = H // 128            # 8 contraction chunks over H
TOK = B * T


import os
NO_CC = bool(int(os.environ.get("EF_NO_CC", "0")))
# Ablation ladder (timing only; results are garbage for GUT != 0):
# 1: per-step x-load only          2: + proj matmuls + ew + h store
# 3: + AllGather + hT load         4: + rec matmuls
# 5: full minus collective (static hT reload each step)
# 6: empty loop (tail only)        7: per-step quarter-size x-load
# 8: 4-step-batched x-load         10: per-step flat 16-descriptor load
GUT = int(os.environ.get("EF_GUT", "0"))


def build_kernel(t_steps=T):
    nc = bacc.Bacc("TRN2", target_bir_lowering=False, debug=False,
                   num_devices=N_CORES)

    # x arrives T-sharded (each core holds t_steps/8 steps); the full xT is
    # rebuilt on-device with one AllGather — host->device staging is the
    # dominant per-call cost, so ship 1/8th of the bytes.
    t_sh = t_steps // N_CORES
    xTs = nc.dram_tensor("xTs", [t_sh, 128, NKX, B], BF16,
                         kind="ExternalInput")
    wihT = nc.dram_tensor("wihT", [NKX, 128, GSL], BF16, kind="ExternalInput")
    whhT = nc.dram_tensor("whhT", [NKH, 128, GSL], BF16, kind="ExternalInput")
    fc1wT = nc.dram_tensor("fc1wT", [NKH, 128, 128], BF16, kind="ExternalInput")
    fc1b = nc.dram_tensor("fc1b", [128, 1], F32, kind="ExternalInput")
    fc2wT = nc.dram_tensor("fc2wT", [128, O], BF16, kind="ExternalInput")
    fc2b = nc.dram_tensor("fc2b", [128, O], F32, kind="ExternalInput")

    n_tok_loc = B * t_steps // N_CORES
    out_sh = nc.dram_tensor("out_sh", [n_tok_loc, O], F32, kind="ExternalOutput")

    with tile.TileContext(nc) as tc:
        with (
            tc.tile_pool(name="const", bufs=1) as const,
            tc.tile_pool(name="xtp", bufs=3) as xtp,
            tc.tile_pool(name="pg", bufs=2, space="PSUM") as pgp,
            tc.tile_pool(name="ph", bufs=2, space="PSUM") as php,
            tc.tile_pool(name="pf", bufs=2, space="PSUM") as pfp,
            tc.tile_pool(name="ew", bufs=2) as ewp,
            tc.tile_pool(name="htp", bufs=3) as htp,
            tc.tile_pool(name="dram", bufs=2, space="DRAM") as dramp,
            tc.tile_pool(name="dram1", bufs=1, space="DRAM") as dramp1,
        ):
            # ---- resident weights ----
            wih_sb = const.tile([128, NKX, GSL], BF16)
            nc.sync.dma_start(wih_sb[:], wihT[:].rearrange("k p g -> p k g"))
            whh_sb = const.tile([128, NKH, GSL], BF16)
            nc.sync.dma_start(whh_sb[:], whhT[:].rearrange("k p g -> p k g"))
            fc1_sb = const.tile([128, NKH, 128], BF16)
            nc.sync.dma_start(fc1_sb[:], fc1wT[:].rearrange("k p c -> p k c"))
            fc1b_sb = const.tile([128, 1], F32)
            nc.sync.dma_start(fc1b_sb[:], fc1b[:])
            fc2w_sb = const.tile([128, O], BF16)
            nc.sync.dma_start(fc2w_sb[:], fc2wT[:])
            fc2b_sb = const.tile([128, O], F32)
            nc.sync.dma_start(fc2b_sb[:], fc2b[:])

            c_sb = const.tile([128, 128], F32)   # persistent cell state slice
            # head partials accumulate in SBUF; one bulk store after the loop
            out_acc = const.tile([128, t_steps, O], F32)

            out_part = dramp1.tile([n_tok_loc * N_CORES, O], F32)

            # ---- rebuild full xT on-device: copy shard to an internal DRAM
            # tile (collectives cannot touch kernel I/O), then AllGather ----
            x_stage = dramp1.tile([t_sh * 128, NKX * B], BF16)
            nc.sync.dma_start(
                x_stage[:], xTs[:].rearrange("t p k b -> (t p) (k b)"))
            xT_full = dramp1.tile([t_steps * 128, NKX * B], BF16,
                                  addr_space="Shared")
            nc.gpsimd.collective_compute(
                "AllGather",
                ALU.bypass,
                replica_groups=[list(range(N_CORES))],
                ins=[x_stage[:].opt()],
                outs=[xT_full[:].opt()],
            )
            xT = xT_full[:].rearrange("(t p) n -> t p n", p=128).rearrange(
                "t p (k b) -> t p k b", k=NKX)

            hT_prev = None
            gate_slices = [(mc * 128, (mc + 1) * 128) for mc in range(4)]

            # GUT=5: static DRAM h buffer, written once, re-loaded per step
            stat_h = None
            if GUT == 5:
                stat_h = dramp1.tile([H, B], BF16)
            if GUT not in (0, 5):
                nc.vector.memset(out_acc[:], 0.0)

            do_proj = GUT not in (1, 6, 7, 8, 10)
            do_rec = GUT in (0, 4, 5)
            do_head = GUT in (0, 5)
            do_ew = GUT not in (1, 6, 7, 8, 10)
            do_store = GUT not in (1, 6, 7, 8, 10)
            do_ag = GUT in (0, 3)
            do_load = GUT in (0, 3, 4, 5)

            for t in range(t_steps):
                # -- x load for step t (ablation variants change its shape) --
                if GUT == 6:
                    pass
                elif GUT == 7:
                    xts = xtp.tile([128, 1, B], BF16)
                    nc.sync.dma_start(xts[:], xT[t][:, 0:1, :])
                elif GUT == 8:
                    if t % 4 == 0:
                        xts = xtp.tile([128, 4, NKX, B], BF16)
                        nc.sync.dma_start(
                            xts[:],
                            xT[t:t + 4].rearrange("t p k b -> p t k b"))
                elif GUT == 10:
                    xts = xtp.tile([8, 128 * NKX * B // 8], BF16)
                    nc.sync.dma_start(
                        xts[:],
                        xT[t].rearrange("p k b -> (p k b)").rearrange(
                            "(q n) -> q n", q=8))
                else:
                    xts = xtp.tile([128, NKX, B], BF16)
                    nc.sync.dma_start(xts[:], xT[t])
                if do_proj:
                    pg = pgp.tile([128, GSL], F32)
                    for mc in range(4):
                        lo, hi = gate_slices[mc]
                        for kc in range(NKX):
                            kk = KC_X[kc]
                            nc.tensor.matmul(
                                pg[:, lo:hi],
                                wih_sb[:kk, kc, lo:hi],
                                xts[:kk, kc, :],
                                start=(kc == 0),
                                stop=((t == 0 or not do_rec)
                                      and kc == NKX - 1),
                            )
                # -- recurrent matmuls (consume previous gathered h) --
                if do_rec and t > 0:
                    for mc in range(4):
                        lo, hi = gate_slices[mc]
                        for kc in range(NKH):
                            nc.tensor.matmul(
                                pg[:, lo:hi],
                                whh_sb[:, kc, lo:hi],
                                hT_prev[:, kc, :],
                                start=False,
                                stop=(kc == NKH - 1),
                            )

                # -- head for step t-1 (emitted after rec so PE prioritizes rec) --
                if do_head and t > 0:
                    emit_head(nc, php, pfp, ewp, fc1_sb, fc1b_sb, fc2w_sb,
                              hT_prev, out_acc, t - 1)

                # -- elementwise: gates -> h slice (i,f,g,o gate chunk order;
                #    c-update runs while the o-gate matmuls finish) --
                if do_ew:
                    first = t == 0 or not do_rec
                    gnl = ewp.tile([128, GSL], F32)
                    nc.scalar.activation(gnl[:, 0:256], pg[:, 0:256], AF.Sigmoid)
                    if not first:
                        fcs = ewp.tile([128, 128], F32)
                        nc.vector.tensor_mul(fcs[:], gnl[:, 128:256], c_sb[:])
                    nc.scalar.activation(gnl[:, 256:384], pg[:, 256:384], AF.Tanh)
                    ig = ewp.tile([128, 128], F32)
                    nc.vector.tensor_mul(ig[:], gnl[:, 0:128], gnl[:, 256:384])
                    if first:
                        nc.vector.tensor_copy(c_sb[:], ig[:])
                    else:
                        nc.vector.tensor_add(c_sb[:], fcs[:], ig[:])
                    tc_t = ewp.tile([128, 128], F32)
                    nc.scalar.activation(tc_t[:], c_sb[:], AF.Tanh)
                    nc.scalar.activation(gnl[:, 384:512], pg[:, 384:512],
                                         AF.Sigmoid)
                    h_sl = ewp.tile([128, 128], BF16)
                    nc.vector.tensor_mul(h_sl[:], gnl[:, 384:512], tc_t[:])

                # -- exchange: AllGather h_T across the 8 cores --
                if do_store:
                    bnc_in = dramp.tile([128, B], BF16)
                    nc.sync.dma_start(bnc_in[:], h_sl[:])
                if GUT == 5 and t == 0:
                    for kc in range(NKH):
                        nc.sync.dma_start(
                            stat_h[kc * 128:(kc + 1) * 128, :], h_sl[:])
                if do_ag:
                    bnc_out = dramp.tile(
                        [H, B], BF16,
                        addr_space="Local" if NO_CC else "Shared")
                    if NO_CC:
                        for kc in range(NKH):
                            nc.gpsimd.dma_start(
                                bnc_out[kc * 128:(kc + 1) * 128, :], bnc_in[:])
                    else:
                        nc.gpsimd.collective_compute(
                            "AllGather",
                            ALU.bypass,
                            replica_groups=[list(range(N_CORES))],
                            ins=[bnc_in[:].opt()],
                            outs=[bnc_out[:].opt()],
                        )
                elif do_load:
                    bnc_out = stat_h if GUT == 5 else dramp.tile([H, B], BF16)
                if do_load:
                    hT = htp.tile([128, NKH, B], BF16)
                    nc.sync.dma_start(
                        hT[:], bnc_out[:].rearrange("(k p) b -> p k b", p=128))
                    hT_prev = hT

            # head for the final step
            if do_head:
                emit_head(nc, php, pfp, ewp, fc1_sb, fc1b_sb, fc2w_sb,
                          hT_prev, out_acc, t_steps - 1)

            # bulk store of the accumulated head partials (token-major)
            nc.sync.dma_start(
                out_part[:].rearrange("(b t) o -> b t o", t=t_steps),
                out_acc[:])

            # ---- tail: ReduceScatter fc2 partials, bias + log_softmax ----
            rs_out = dramp1.tile([n_tok_loc, O], F32)
            nc.gpsimd.collective_compute(
                "ReduceScatter",
                ALU.add,
                replica_groups=[list(range(N_CORES))],
                ins=[out_part[:].opt()],
                outs=[rs_out[:].opt()],
            )
            # token rows viewed as [128, CH, O]: partition p owns tokens
            # p*CH..(p+1)*CH-1 — contiguous per-partition DMA lines
            CH = n_tok_loc // 128
            z_sb = ewp.tile([128, CH, O], F32)
            nc.sync.dma_start(
                z_sb[:], rs_out[:].rearrange("(p c) o -> p c o", p=128))
            res_sb = ewp.tile([128, CH, O], F32)
            for ch in range(CH):
                zb = ewp.tile([128, O], F32)
                nc.vector.tensor_add(zb[:], z_sb[:, ch, :], fc2b_sb[:])
                mx = ewp.tile([128, 1], F32)
                nc.vector.reduce_max(mx[:], zb[:], axis=mybir.AxisListType.X)
                sh = ewp.tile([128, O], F32)
                nc.vector.tensor_scalar_sub(sh[:], zb[:], mx[:])
                ex = ewp.tile([128, O], F32)
                nc.scalar.activation(ex[:], sh[:], AF.Exp)
                sm = ewp.tile([128, 1], F32)
                nc.vector.reduce_sum(sm[:], ex[:], axis=mybir.AxisListType.X)
                lg = ewp.tile([128, 1], F32)
                nc.scalar.activation(lg[:], sm[:], AF.Ln)
                nc.vector.tensor_scalar_sub(res_sb[:, ch, :], sh[:], lg[:])
            nc.sync.dma_start(
                out_sh[:].rearrange("(p c) o -> p c o", p=128), res_sb[:])

    nc.compile()
    return nc


def emit_head(nc, php, pfp, ewp, fc1_sb, fc1b_sb, fc2w_sb, hT, out_acc, t):
    """fc1 (C slice) + relu + fc2 partial for timestep t → SBUF accumulator."""
    ph = php.tile([128, B], F32)
    for kc in range(NKH):
        nc.tensor.matmul(ph[:], fc1_sb[:, kc, :], hT[:, kc, :],
                         start=(kc == 0), stop=(kc == NKH - 1))
    hid = ewp.tile([128, B], BF16)
    nc.scalar.activation(hid[:], ph[:], AF.Relu, bias=fc1b_sb[:])
    pf = pfp.tile([128, O], F32)
    nc.tensor.matmul(pf[:], hid[:], fc2w_sb[:], start=True, stop=True)
    nc.vector.tensor_copy(out_acc[:, t, :], pf[:])


_CACHED = {}


def _get_kernel(t_steps):
    if t_steps not in _CACHED:
        _CACHED[t_steps] = build_kernel(t_steps)
    return _CACHED[t_steps]


def prep_inputs(m_text, m_audio, m_video, W_ih, W_hh, b_ih, b_hh,
                fc1_w, fc1_b, fc2_w, fc2_b, t_steps=T):
    """Host-side layout prep; returns per-core input maps."""
    bf = ml_dtypes.bfloat16
    x = np.concatenate([np.asarray(m_text), np.asarray(m_audio),
                        np.asarray(m_video)], axis=-1).astype(np.float32)
    b_, t_, d_ = x.shape
    assert (b_, d_) == (B, D) and t_ == t_steps
    # x_T: [T, D+1(ones) padded to 4*128, B], stored [t, p, k, b] so the
    # per-step SBUF load is one contiguous 1KB line per partition
    xTf = np.zeros((t_steps, NKX * 128, B), np.float32)
    xTf[:, :D, :] = x.transpose(1, 2, 0)
    xTf[:, D, :] = 1.0
    xTf = np.ascontiguousarray(
        xTf.reshape(t_steps, NKX, 128, B).transpose(0, 2, 1, 3)).astype(bf)

    W_ih = np.asarray(W_ih, np.float32)
    W_hh = np.asarray(W_hh, np.float32)
    bias = (np.asarray(b_ih) + np.asarray(b_hh)).astype(np.float32)
    fc1_w = np.asarray(fc1_w, np.float32)
    fc1_b = np.asarray(fc1_b, np.float32)
    fc2_w = np.asarray(fc2_w, np.float32)
    fc2_b = np.asarray(fc2_b, np.float32)

    in_maps = []
    gate_order = (0, 1, 2, 3)  # i, f, g, o (pytorch row-block order)
    for k in range(N_CORES):
        rows = np.concatenate(
            [np.arange(g * H + k * HSL, g * H + (k + 1) * HSL)
             for g in gate_order])
        wih_sl = W_ih[rows, :]            # [512, 409]
        whh_sl = W_hh[rows, :]            # [512, 1024]
        b_sl = bias[rows]                 # [512]
        wihT_k = np.zeros((NKX * 128, GSL), np.float32)
        wihT_k[:D, :] = wih_sl.T
        wihT_k[D, :] = b_sl
        wihT_k = np.ascontiguousarray(
            wihT_k.reshape(NKX, 128, GSL)).astype(bf)
        whhT_k = np.ascontiguousarray(
            whh_sl.T.reshape(NKH, 128, GSL)).astype(bf)

        cc = k % 4                        # C chunk (cores k and k+4 duplicate)
        crows = np.arange(cc * 128, (cc + 1) * 128)
        fc1wT_k = np.ascontiguousarray(
            fc1_w[crows, :].T.reshape(NKH, 128, 128)).astype(bf)
        fc1b_k = np.ascontiguousarray(fc1_b[crows].reshape(128, 1))
        fc2wT_k = np.ascontiguousarray(0.5 * fc2_w[:, crows].T).astype(bf)
        fc2b_k = np.ascontiguousarray(
            np.broadcast_to(fc2_b[None, :], (128, O))).astype(np.float32)

        t_sh = t_steps // N_CORES
        in_maps.append({
            "xTs": xTf[k * t_sh:(k + 1) * t_sh],
            "wihT": wihT_k,
            "whhT": whhT_k,
            "fc1wT": fc1wT_k,
            "fc1b": fc1b_k,
            "fc2wT": fc2wT_k,
            "fc2b": fc2b_k,
        })
    return in_maps


def run(inputs, t_steps=T, trace=False):
    nc = _get_kernel(t_steps)
    in_maps = prep_inputs(
        inputs["m_text"], inputs["m_audio"], inputs["m_video"],
        inputs["W_ih"], inputs["W_hh"], inputs["b_ih"], inputs["b_hh"],
        inputs["fc1_w"], inputs["fc1_b"], inputs["fc2_w"], inputs["fc2_b"],
        t_steps=t_steps)
    res = run_bass_kernel_spmd(
        nc, in_maps, core_ids=list(range(N_CORES)), trace=trace)
    shards = [res.results[k]["out_sh"] for k in range(N_CORES)]
    full = np.concatenate(shards, axis=0)          # [(b t), O] token-major
    out = full.reshape(B, t_steps, O)
    return out, res


def kernel(**inputs) -> np.ndarray:
    t_steps = np.asarray(inputs["m_text"]).shape[1]
    out, _ = run(inputs, t_steps=t_steps)
    return out.astype(np.float32)



# revision 23
# speedup vs baseline: 3.0366x; 2.1531x over previous
"""EFLSTM Trainium2 kernel: 8-core tensor-parallel LSTM + fused head.

Strategy (8 NeuronCores, SPMD single program, per-core data differs):
  - Tensor-parallel over the 4H gate dimension: core k owns gate rows
    [k*128:(k+1)*128] of each of the four gates (i, f, g, o), so it computes
    h rows [k*128:(k+1)*128] each timestep.  A per-step AllGather rebuilds
    the full transposed hidden state h_T [H=1024, B=128] on every core.
  - Input projection x @ W_ih.T is folded into the recurrent PSUM
    accumulation (x is pre-transposed on the host with a ones-row so the
    bias is folded into the matmul as well).  The projection matmuls do not
    depend on h, so they execute during the AllGather latency window.
  - Matmul operands and the h exchange are bf16 (fp32 PSUM accumulation,
    fp32 cell state); gate nonlinearities are split per gate chunk so the
    c-update overlaps the o-gate matmuls.
  - FC head is fused per-step: fc1 (one 128-wide slice of C per core; cores
    k and k+4 duplicate a slice, fc2 weights pre-scaled by 0.5 so the final
    ReduceScatter sum is correct) + fc2 partials accumulate into a
    token-major buffer; after the loop one ReduceScatter + log_softmax on
    each core's token shard produces the output.
"""

import numpy as np
import ml_dtypes

import concourse.bacc as bacc
import concourse.mybir as mybir
import concourse.tile as tile
from concourse.bass_utils import run_bass_kernel_spmd

F32 = mybir.dt.float32
BF16 = mybir.dt.bfloat16
FP8 = mybir.dt.float8e4
AF = mybir.ActivationFunctionType
ALU = mybir.AluOpType

N_CORES = 8
B = 128
T = 512
DIMS = (300, 74, 35)
D = sum(DIMS)  # 409
DP = D + 1     # 410: extra ones-row folds the gate bias into the matmul
H = 1024
G = 4 * H
C = 512
O = 7
HSL = H // N_CORES        # 128 h rows per core
GSL = 4 * HSL             # 512 gate rows per core
KC_X = [128, 128, 128, DP - 3 * 128]   # contraction chunks over D+1
NKX = len(KC_X)
NKH = H // 128            # 8 contraction chunks over H
TOK = B * T


import os
NO_CC = bool(int(os.environ.get("EF_NO_CC", "0")))
# Ablation ladder (timing only; results are garbage for GUT != 0):
# 1: per-step x-load only          2: + proj matmuls + ew + h store
# 3: + AllGather + hT load         4: + rec matmuls
# 5: full minus collective (static hT reload each step)
# 6: empty loop (tail only)        7: per-step quarter-size x-load
# 8: 4-step-batched x-load         10: per-step flat 16-descriptor load
GUT = int(os.environ.get("EF_GUT", "0"))


def build_kernel(t_steps=T):
    nc = bacc.Bacc("TRN2", target_bir_lowering=False, debug=False,
                   num_devices=N_CORES)

    # x arrives T-sharded (each core holds t_steps/8 steps); the full xT is
    # rebuilt on-device with one AllGather — host->device staging is the
    # dominant per-call cost, so ship 1/8th of the bytes.
    t_sh = t_steps // N_CORES
    xTs = nc.dram_tensor("xTs", [t_sh, 128, NKX, B], FP8,
                         kind="ExternalInput")
    wihT = nc.dram_tensor("wihT", [NKX, 128, GSL], BF16, kind="ExternalInput")
    whhT = nc.dram_tensor("whhT", [NKH, 128, GSL], BF16, kind="ExternalInput")
    fc1wT = nc.dram_tensor("fc1wT", [NKH, 128, 128], BF16, kind="ExternalInput")
    fc1b = nc.dram_tensor("fc1b", [128, 1], F32, kind="ExternalInput")
    fc2wT = nc.dram_tensor("fc2wT", [128, O], BF16, kind="ExternalInput")
    fc2b = nc.dram_tensor("fc2b", [128, O], F32, kind="ExternalInput")

    n_tok_loc = B * t_steps // N_CORES
    out_sh = nc.dram_tensor("out_sh", [n_tok_loc, O], F32, kind="ExternalOutput")

    with tile.TileContext(nc) as tc:
        with (
            tc.tile_pool(name="const", bufs=1) as const,
            tc.tile_pool(name="xtp", bufs=3) as xtp,
            tc.tile_pool(name="pg", bufs=2, space="PSUM") as pgp,
            tc.tile_pool(name="ph", bufs=2, space="PSUM") as php,
            tc.tile_pool(name="pf", bufs=2, space="PSUM") as pfp,
            tc.tile_pool(name="ew", bufs=2) as ewp,
            tc.tile_pool(name="htp", bufs=3) as htp,
            tc.tile_pool(name="dram", bufs=2, space="DRAM") as dramp,
            tc.tile_pool(name="dram1", bufs=1, space="DRAM") as dramp1,
        ):
            # ---- resident weights ----
            wih_sb = const.tile([128, NKX, GSL], BF16)
            nc.sync.dma_start(wih_sb[:], wihT[:].rearrange("k p g -> p k g"))
            whh_sb = const.tile([128, NKH, GSL], BF16)
            nc.sync.dma_start(whh_sb[:], whhT[:].rearrange("k p g -> p k g"))
            fc1_sb = const.tile([128, NKH, 128], BF16)
            nc.sync.dma_start(fc1_sb[:], fc1wT[:].rearrange("k p c -> p k c"))
            fc1b_sb = const.tile([128, 1], F32)
            nc.sync.dma_start(fc1b_sb[:], fc1b[:])
            fc2w_sb = const.tile([128, O], BF16)
            nc.sync.dma_start(fc2w_sb[:], fc2wT[:])
            fc2b_sb = const.tile([128, O], F32)
            nc.sync.dma_start(fc2b_sb[:], fc2b[:])

            c_sb = const.tile([128, 128], F32)   # persistent cell state slice
            # head partials accumulate in SBUF; one bulk store after the loop
            out_acc = const.tile([128, t_steps, O], F32)

            out_part = dramp1.tile([n_tok_loc * N_CORES, O], F32)

            # ---- rebuild full xT on-device: copy shard to an internal DRAM
            # tile (collectives cannot touch kernel I/O), then AllGather ----
            x_stage = dramp1.tile([t_sh * 128, NKX * B], FP8)
            nc.sync.dma_start(
                x_stage[:], xTs[:].rearrange("t p k b -> (t p) (k b)"))
            xT_full = dramp1.tile([t_steps * 128, NKX * B], FP8,
                                  addr_space="Shared")
            nc.gpsimd.collective_compute(
                "AllGather",
                ALU.bypass,
                replica_groups=[list(range(N_CORES))],
                ins=[x_stage[:].opt()],
                outs=[xT_full[:].opt()],
            )
            xT = xT_full[:].rearrange("(t p) n -> t p n", p=128).rearrange(
                "t p (k b) -> t p k b", k=NKX)

            hT_prev = None
            gate_slices = [(mc * 128, (mc + 1) * 128) for mc in range(4)]

            # GUT=5: static DRAM h buffer, written once, re-loaded per step
            stat_h = None
            if GUT == 5:
                stat_h = dramp1.tile([H, B], BF16)
            if GUT not in (0, 5):
                nc.vector.memset(out_acc[:], 0.0)

            do_proj = GUT not in (1, 6, 7, 8, 10)
            do_rec = GUT in (0, 4, 5)
            do_head = GUT in (0, 5)
            do_ew = GUT not in (1, 6, 7, 8, 10)
            do_store = GUT not in (1, 6, 7, 8, 10)
            do_ag = GUT in (0, 3)
            do_load = GUT in (0, 3, 4, 5)

            for t in range(t_steps):
                # -- x load for step t (ablation variants change its shape) --
                if GUT == 6:
                    pass
                elif GUT == 7:
                    xts = xtp.tile([128, 1, B], FP8)
                    nc.sync.dma_start(xts[:], xT[t][:, 0:1, :])
                elif GUT == 8:
                    if t % 4 == 0:
                        xts = xtp.tile([128, 4, NKX, B], FP8)
                        nc.sync.dma_start(
                            xts[:],
                            xT[t:t + 4].rearrange("t p k b -> p t k b"))
                elif GUT == 10:
                    xts = xtp.tile([8, 128 * NKX * B // 8], FP8)
                    nc.sync.dma_start(
                        xts[:],
                        xT[t].rearrange("p k b -> (p k b)").rearrange(
                            "(q n) -> q n", q=8))
                else:
                    # SWDGE cast-DMA: fp8 in DRAM -> bf16 in SBUF
                    xts = xtp.tile([128, NKX, B], BF16)
                    nc.gpsimd.dma_start(xts[:], xT[t])
                if do_proj:
                    pg = pgp.tile([128, GSL], F32)
                    for mc in range(4):
                        lo, hi = gate_slices[mc]
                        for kc in range(NKX):
                            kk = KC_X[kc]
                            nc.tensor.matmul(
                                pg[:, lo:hi],
                                wih_sb[:kk, kc, lo:hi],
                                xts[:kk, kc, :],
                                start=(kc == 0),
                                stop=((t == 0 or not do_rec)
                                      and kc == NKX - 1),
                            )
                # -- recurrent matmuls (consume previous gathered h) --
                if do_rec and t > 0:
                    for mc in range(4):
                        lo, hi = gate_slices[mc]
                        for kc in range(NKH):
                            nc.tensor.matmul(
                                pg[:, lo:hi],
                                whh_sb[:, kc, lo:hi],
                                hT_prev[:, kc, :],
                                start=False,
                                stop=(kc == NKH - 1),
                            )

                # -- head for step t-1 (emitted after rec so PE prioritizes rec) --
                if do_head and t > 0:
                    emit_head(nc, php, pfp, ewp, fc1_sb, fc1b_sb, fc2w_sb,
                              hT_prev, out_acc, t - 1)

                # -- elementwise: gates -> h slice (i,f,g,o gate chunk order;
                #    c-update runs while the o-gate matmuls finish) --
                if do_ew:
                    first = t == 0 or not do_rec
                    gnl = ewp.tile([128, GSL], F32)
                    nc.scalar.activation(gnl[:, 0:256], pg[:, 0:256], AF.Sigmoid)
                    if not first:
                        fcs = ewp.tile([128, 128], F32)
                        nc.vector.tensor_mul(fcs[:], gnl[:, 128:256], c_sb[:])
                    nc.scalar.activation(gnl[:, 256:384], pg[:, 256:384], AF.Tanh)
                    ig = ewp.tile([128, 128], F32)
                    nc.vector.tensor_mul(ig[:], gnl[:, 0:128], gnl[:, 256:384])
                    if first:
                        nc.vector.tensor_copy(c_sb[:], ig[:])
                    else:
                        nc.vector.tensor_add(c_sb[:], fcs[:], ig[:])
                    tc_t = ewp.tile([128, 128], F32)
                    nc.scalar.activation(tc_t[:], c_sb[:], AF.Tanh)
                    nc.scalar.activation(gnl[:, 384:512], pg[:, 384:512],
                                         AF.Sigmoid)
                    h_sl = ewp.tile([128, 128], BF16)
                    nc.vector.tensor_mul(h_sl[:], gnl[:, 384:512], tc_t[:])

                # -- exchange: AllGather h_T across the 8 cores --
                if do_store:
                    bnc_in = dramp.tile([128, B], BF16)
                    nc.sync.dma_start(bnc_in[:], h_sl[:])
                if GUT == 5 and t == 0:
                    for kc in range(NKH):
                        nc.sync.dma_start(
                            stat_h[kc * 128:(kc + 1) * 128, :], h_sl[:])
                if do_ag:
                    bnc_out = dramp.tile(
                        [H, B], BF16,
                        addr_space="Local" if NO_CC else "Shared")
                    if NO_CC:
                        for kc in range(NKH):
                            nc.gpsimd.dma_start(
                                bnc_out[kc * 128:(kc + 1) * 128, :], bnc_in[:])
                    else:
                        nc.gpsimd.collective_compute(
                            "AllGather",
                            ALU.bypass,
                            replica_groups=[list(range(N_CORES))],
                            ins=[bnc_in[:].opt()],
                            outs=[bnc_out[:].opt()],
                        )
                elif do_load:
                    bnc_out = stat_h if GUT == 5 else dramp.tile([H, B], BF16)
                if do_load:
                    hT = htp.tile([128, NKH, B], BF16)
                    nc.sync.dma_start(
                        hT[:], bnc_out[:].rearrange("(k p) b -> p k b", p=128))
                    hT_prev = hT

            # head for the final step
            if do_head:
                emit_head(nc, php, pfp, ewp, fc1_sb, fc1b_sb, fc2w_sb,
                          hT_prev, out_acc, t_steps - 1)

            # bulk store of the accumulated head partials (token-major)
            nc.sync.dma_start(
                out_part[:].rearrange("(b t) o -> b t o", t=t_steps),
                out_acc[:])

            # ---- tail: ReduceScatter fc2 partials, bias + log_softmax ----
            rs_out = dramp1.tile([n_tok_loc, O], F32)
            nc.gpsimd.collective_compute(
                "ReduceScatter",
                ALU.add,
                replica_groups=[list(range(N_CORES))],
                ins=[out_part[:].opt()],
                outs=[rs_out[:].opt()],
            )
            # token rows viewed as [128, CH, O]: partition p owns tokens
            # p*CH..(p+1)*CH-1 — contiguous per-partition DMA lines
            CH = n_tok_loc // 128
            z_sb = ewp.tile([128, CH, O], F32)
            nc.sync.dma_start(
                z_sb[:], rs_out[:].rearrange("(p c) o -> p c o", p=128))
            res_sb = ewp.tile([128, CH, O], F32)
            for ch in range(CH):
                zb = ewp.tile([128, O], F32)
                nc.vector.tensor_add(zb[:], z_sb[:, ch, :], fc2b_sb[:])
                mx = ewp.tile([128, 1], F32)
                nc.vector.reduce_max(mx[:], zb[:], axis=mybir.AxisListType.X)
                sh = ewp.tile([128, O], F32)
                nc.vector.tensor_scalar_sub(sh[:], zb[:], mx[:])
                ex = ewp.tile([128, O], F32)
                nc.scalar.activation(ex[:], sh[:], AF.Exp)
                sm = ewp.tile([128, 1], F32)
                nc.vector.reduce_sum(sm[:], ex[:], axis=mybir.AxisListType.X)
                lg = ewp.tile([128, 1], F32)
                nc.scalar.activation(lg[:], sm[:], AF.Ln)
                nc.vector.tensor_scalar_sub(res_sb[:, ch, :], sh[:], lg[:])
            nc.sync.dma_start(
                out_sh[:].rearrange("(p c) o -> p c o", p=128), res_sb[:])

    nc.compile()
    return nc


def emit_head(nc, php, pfp, ewp, fc1_sb, fc1b_sb, fc2w_sb, hT, out_acc, t):
    """fc1 (C slice) + relu + fc2 partial for timestep t → SBUF accumulator."""
    ph = php.tile([128, B], F32)
    for kc in range(NKH):
        nc.tensor.matmul(ph[:], fc1_sb[:, kc, :], hT[:, kc, :],
                         start=(kc == 0), stop=(kc == NKH - 1))
    hid = ewp.tile([128, B], BF16)
    nc.scalar.activation(hid[:], ph[:], AF.Relu, bias=fc1b_sb[:])
    pf = pfp.tile([128, O], F32)
    nc.tensor.matmul(pf[:], hid[:], fc2w_sb[:], start=True, stop=True)
    nc.vector.tensor_copy(out_acc[:, t, :], pf[:])


_CACHED = {}


def _get_kernel(t_steps):
    if t_steps not in _CACHED:
        _CACHED[t_steps] = build_kernel(t_steps)
    return _CACHED[t_steps]


def prep_inputs(m_text, m_audio, m_video, W_ih, W_hh, b_ih, b_hh,
                fc1_w, fc1_b, fc2_w, fc2_b, t_steps=T):
    """Host-side layout prep; returns per-core input maps."""
    bf = ml_dtypes.bfloat16
    x = np.concatenate([np.asarray(m_text), np.asarray(m_audio),
                        np.asarray(m_video)], axis=-1).astype(np.float32)
    b_, t_, d_ = x.shape
    assert (b_, d_) == (B, D) and t_ == t_steps
    # x_T: [T, D+1(ones) padded to 4*128, B], stored [t, p, k, b] so the
    # per-step SBUF load is one contiguous 1KB line per partition
    xTf = np.zeros((t_steps, NKX * 128, B), np.float32)
    xTf[:, :D, :] = x.transpose(1, 2, 0)
    xTf[:, D, :] = 1.0
    xTf = np.ascontiguousarray(
        xTf.reshape(t_steps, NKX, 128, B).transpose(0, 2, 1, 3)).astype(
            ml_dtypes.float8_e4m3)

    W_ih = np.asarray(W_ih, np.float32)
    W_hh = np.asarray(W_hh, np.float32)
    bias = (np.asarray(b_ih) + np.asarray(b_hh)).astype(np.float32)
    fc1_w = np.asarray(fc1_w, np.float32)
    fc1_b = np.asarray(fc1_b, np.float32)
    fc2_w = np.asarray(fc2_w, np.float32)
    fc2_b = np.asarray(fc2_b, np.float32)

    in_maps = []
    gate_order = (0, 1, 2, 3)  # i, f, g, o (pytorch row-block order)
    for k in range(N_CORES):
        rows = np.concatenate(
            [np.arange(g * H + k * HSL, g * H + (k + 1) * HSL)
             for g in gate_order])
        wih_sl = W_ih[rows, :]            # [512, 409]
        whh_sl = W_hh[rows, :]            # [512, 1024]
        b_sl = bias[rows]                 # [512]
        wihT_k = np.zeros((NKX * 128, GSL), np.float32)
        wihT_k[:D, :] = wih_sl.T
        wihT_k[D, :] = b_sl
        wihT_k = np.ascontiguousarray(
            wihT_k.reshape(NKX, 128, GSL)).astype(bf)
        whhT_k = np.ascontiguousarray(
            whh_sl.T.reshape(NKH, 128, GSL)).astype(bf)

        cc = k % 4                        # C chunk (cores k and k+4 duplicate)
        crows = np.arange(cc * 128, (cc + 1) * 128)
        fc1wT_k = np.ascontiguousarray(
            fc1_w[crows, :].T.reshape(NKH, 128, 128)).astype(bf)
        fc1b_k = np.ascontiguousarray(fc1_b[crows].reshape(128, 1))
        fc2wT_k = np.ascontiguousarray(0.5 * fc2_w[:, crows].T).astype(bf)
        fc2b_k = np.ascontiguousarray(
            np.broadcast_to(fc2_b[None, :], (128, O))).astype(np.float32)

        t_sh = t_steps // N_CORES
        in_maps.append({
            "xTs": xTf[k * t_sh:(k + 1) * t_sh],
            "wihT": wihT_k,
            "whhT": whhT_k,
            "fc1wT": fc1wT_k,
            "fc1b": fc1b_k,
            "fc2wT": fc2wT_k,
            "fc2b": fc2b_k,
        })
    return in_maps


def run(inputs, t_steps=T, trace=False):
    nc = _get_kernel(t_steps)
    in_maps = prep_inputs(
        inputs["m_text"], inputs["m_audio"], inputs["m_video"],
        inputs["W_ih"], inputs["W_hh"], inputs["b_ih"], inputs["b_hh"],
        inputs["fc1_w"], inputs["fc1_b"], inputs["fc2_w"], inputs["fc2_b"],
        t_steps=t_steps)
    res = run_bass_kernel_spmd(
        nc, in_maps, core_ids=list(range(N_CORES)), trace=trace)
    shards = [res.results[k]["out_sh"] for k in range(N_CORES)]
    full = np.concatenate(shards, axis=0)          # [(b t), O] token-major
    out = full.reshape(B, t_steps, O)
    return out, res


def kernel(**inputs) -> np.ndarray:
    t_steps = np.asarray(inputs["m_text"]).shape[1]
    out, _ = run(inputs, t_steps=t_steps)
    return out.astype(np.float32)



# revision 28
# speedup vs baseline: 3.1087x; 1.0237x over previous
"""EFLSTM Trainium2 kernel: 8-core tensor-parallel LSTM + fused head.

Strategy (8 NeuronCores, SPMD single program, per-core data differs):
  - Tensor-parallel over the 4H gate dimension: core k owns gate rows
    [k*128:(k+1)*128] of each of the four gates (i, f, g, o), so it computes
    h rows [k*128:(k+1)*128] each timestep.  A per-step AllGather rebuilds
    the full transposed hidden state h_T [H=1024, B=128] on every core.
  - Input projection x @ W_ih.T is folded into the recurrent PSUM
    accumulation (x is pre-transposed on the host with a ones-row so the
    bias is folded into the matmul as well).  The projection matmuls do not
    depend on h, so they execute during the AllGather latency window.
  - Matmul operands and the h exchange are bf16 (fp32 PSUM accumulation,
    fp32 cell state); gate nonlinearities are split per gate chunk so the
    c-update overlaps the o-gate matmuls.
  - FC head is fused per-step: fc1 (one 128-wide slice of C per core; cores
    k and k+4 duplicate a slice, fc2 weights pre-scaled by 0.5 so the final
    ReduceScatter sum is correct) + fc2 partials accumulate into a
    token-major buffer; after the loop one ReduceScatter + log_softmax on
    each core's token shard produces the output.
"""

import numpy as np
import ml_dtypes

import concourse.bacc as bacc
import concourse.mybir as mybir
import concourse.tile as tile
from concourse.bass_utils import run_bass_kernel_spmd

F32 = mybir.dt.float32
BF16 = mybir.dt.bfloat16
FP8 = mybir.dt.float8e4
AF = mybir.ActivationFunctionType
ALU = mybir.AluOpType

N_CORES = 8
B = 128
T = 512
DIMS = (300, 74, 35)
D = sum(DIMS)  # 409
DP = D + 1     # 410: extra ones-row folds the gate bias into the matmul
H = 1024
G = 4 * H
C = 512
O = 7
HSL = H // N_CORES        # 128 h rows per core
GSL = 4 * HSL             # 512 gate rows per core
KC_X = [128, 128, 128, DP - 3 * 128]   # contraction chunks over D+1
NKX = len(KC_X)
NKH = H // 128            # 8 contraction chunks over H
TOK = B * T


import os
NO_CC = bool(int(os.environ.get("EF_NO_CC", "0")))
# Ablation ladder (timing only; results are garbage for GUT != 0):
# 1: per-step x-load only          2: + proj matmuls + ew + h store
# 3: + AllGather + hT load         4: + rec matmuls
# 5: full minus collective (static hT reload each step)
# 6: empty loop (tail only)        7: per-step quarter-size x-load
# 8: 4-step-batched x-load         10: per-step flat 16-descriptor load
GUT = int(os.environ.get("EF_GUT", "0"))


def build_kernel(t_steps=T):
    nc = bacc.Bacc("TRN2", target_bir_lowering=False, debug=False,
                   num_devices=N_CORES)

    # x arrives T-sharded (each core holds t_steps/8 steps); the full xT is
    # rebuilt on-device with one AllGather — host->device staging is the
    # dominant per-call cost, so ship 1/8th of the bytes.
    t_sh = t_steps // N_CORES
    xTs = nc.dram_tensor("xTs", [t_sh, 128, NKX, B], FP8,
                         kind="ExternalInput")
    # all weights in one p-major bf16 blob + one small f32 bias blob: fewer
    # kernel inputs = less per-call binding/staging overhead
    WCOLS = NKX * GSL + NKH * GSL + NKH * 128 + O   # 7175
    wblob = nc.dram_tensor("wblob", [128, WCOLS], BF16, kind="ExternalInput")
    fblob = nc.dram_tensor("fblob", [128, 1 + O], F32, kind="ExternalInput")

    n_tok_loc = B * t_steps // N_CORES
    out_sh = nc.dram_tensor("out_sh", [n_tok_loc, O], F32, kind="ExternalOutput")

    with tile.TileContext(nc) as tc:
        with (
            tc.tile_pool(name="const", bufs=1) as const,
            tc.tile_pool(name="xtp", bufs=3) as xtp,
            tc.tile_pool(name="pg", bufs=2, space="PSUM") as pgp,
            tc.tile_pool(name="ph", bufs=2, space="PSUM") as php,
            tc.tile_pool(name="pf", bufs=2, space="PSUM") as pfp,
            tc.tile_pool(name="ew", bufs=2) as ewp,
            tc.tile_pool(name="htp", bufs=3) as htp,
            tc.tile_pool(name="dram", bufs=2, space="DRAM") as dramp,
            tc.tile_pool(name="dram1", bufs=1, space="DRAM") as dramp1,
        ):
            # ---- resident weights: one contiguous blob load, sliced views --
            w_sb = const.tile([128, WCOLS], BF16)
            nc.sync.dma_start(w_sb[:], wblob[:])
            o_wih = 0
            o_whh = o_wih + NKX * GSL
            o_fc1 = o_whh + NKH * GSL
            o_fc2 = o_fc1 + NKH * 128
            wih_sb = w_sb[:, o_wih:o_whh].rearrange("p (k g) -> p k g", k=NKX)
            whh_sb = w_sb[:, o_whh:o_fc1].rearrange("p (k g) -> p k g", k=NKH)
            fc1_sb = w_sb[:, o_fc1:o_fc2].rearrange("p (k c) -> p k c", k=NKH)
            fc2w_sb = w_sb[:, o_fc2:o_fc2 + O]
            f_sb = const.tile([128, 1 + O], F32)
            nc.sync.dma_start(f_sb[:], fblob[:])
            fc1b_sb = f_sb[:, 0:1]
            fc2b_sb = f_sb[:, 1:1 + O]

            c_sb = const.tile([128, 128], F32)   # persistent cell state slice
            # head partials accumulate in SBUF; one bulk store after the loop
            out_acc = const.tile([128, t_steps, O], F32)

            out_part = dramp1.tile([n_tok_loc * N_CORES, O], F32)

            # ---- rebuild full xT on-device: copy shard to an internal DRAM
            # tile (collectives cannot touch kernel I/O), then AllGather ----
            x_stage = dramp1.tile([t_sh * 128, NKX * B], FP8)
            nc.sync.dma_start(
                x_stage[:], xTs[:].rearrange("t p k b -> (t p) (k b)"))
            xT_full = dramp1.tile([t_steps * 128, NKX * B], FP8,
                                  addr_space="Shared")
            nc.gpsimd.collective_compute(
                "AllGather",
                ALU.bypass,
                replica_groups=[list(range(N_CORES))],
                ins=[x_stage[:].opt()],
                outs=[xT_full[:].opt()],
            )
            xT = xT_full[:].rearrange("(t p) n -> t p n", p=128).rearrange(
                "t p (k b) -> t p k b", k=NKX)

            hT_prev = None
            gate_slices = [(mc * 128, (mc + 1) * 128) for mc in range(4)]

            # GUT=5: static DRAM h buffer, written once, re-loaded per step
            stat_h = None
            if GUT == 5:
                stat_h = dramp1.tile([H, B], BF16)
            if GUT not in (0, 5):
                nc.vector.memset(out_acc[:], 0.0)

            do_proj = GUT not in (1, 6, 7, 8, 10)
            do_rec = GUT in (0, 4, 5)
            do_head = GUT in (0, 5)
            do_ew = GUT not in (1, 6, 7, 8, 10)
            do_store = GUT not in (1, 6, 7, 8, 10)
            do_ag = GUT in (0, 3)
            do_load = GUT in (0, 3, 4, 5)

            for t in range(t_steps):
                # -- x load for step t (ablation variants change its shape) --
                if GUT == 6:
                    pass
                elif GUT == 7:
                    xts = xtp.tile([128, 1, B], FP8)
                    nc.sync.dma_start(xts[:], xT[t][:, 0:1, :])
                elif GUT == 8:
                    if t % 4 == 0:
                        xts = xtp.tile([128, 4, NKX, B], FP8)
                        nc.sync.dma_start(
                            xts[:],
                            xT[t:t + 4].rearrange("t p k b -> p t k b"))
                elif GUT == 10:
                    xts = xtp.tile([8, 128 * NKX * B // 8], FP8)
                    nc.sync.dma_start(
                        xts[:],
                        xT[t].rearrange("p k b -> (p k b)").rearrange(
                            "(q n) -> q n", q=8))
                else:
                    # SWDGE cast-DMA: fp8 in DRAM -> bf16 in SBUF
                    xts = xtp.tile([128, NKX, B], BF16)
                    nc.gpsimd.dma_start(xts[:], xT[t])
                if do_proj:
                    pg = pgp.tile([128, GSL], F32)
                    for mc in range(4):
                        lo, hi = gate_slices[mc]
                        for kc in range(NKX):
                            kk = KC_X[kc]
                            nc.tensor.matmul(
                                pg[:, lo:hi],
                                wih_sb[:kk, kc, lo:hi],
                                xts[:kk, kc, :],
                                start=(kc == 0),
                                stop=((t == 0 or not do_rec)
                                      and kc == NKX - 1),
                            )
                # -- recurrent matmuls (consume previous gathered h) --
                if do_rec and t > 0:
                    for mc in range(4):
                        lo, hi = gate_slices[mc]
                        for kc in range(NKH):
                            nc.tensor.matmul(
                                pg[:, lo:hi],
                                whh_sb[:, kc, lo:hi],
                                hT_prev[:, kc, :],
                                start=False,
                                stop=(kc == NKH - 1),
                            )

                # -- head for step t-1 (emitted after rec so PE prioritizes rec) --
                if do_head and t > 0:
                    emit_head(nc, php, pfp, ewp, fc1_sb, fc1b_sb, fc2w_sb,
                              hT_prev, out_acc, t - 1)

                # -- elementwise: gates -> h slice (i,f,g,o gate chunk order;
                #    c-update runs while the o-gate matmuls finish) --
                if do_ew:
                    first = t == 0 or not do_rec
                    gnl = ewp.tile([128, GSL], F32)
                    nc.scalar.activation(gnl[:, 0:256], pg[:, 0:256], AF.Sigmoid)
                    if not first:
                        fcs = ewp.tile([128, 128], F32)
                        nc.vector.tensor_mul(fcs[:], gnl[:, 128:256], c_sb[:])
                    nc.scalar.activation(gnl[:, 256:384], pg[:, 256:384], AF.Tanh)
                    ig = ewp.tile([128, 128], F32)
                    nc.vector.tensor_mul(ig[:], gnl[:, 0:128], gnl[:, 256:384])
                    if first:
                        nc.vector.tensor_copy(c_sb[:], ig[:])
                    else:
                        nc.vector.tensor_add(c_sb[:], fcs[:], ig[:])
                    tc_t = ewp.tile([128, 128], F32)
                    nc.scalar.activation(tc_t[:], c_sb[:], AF.Tanh)
                    nc.scalar.activation(gnl[:, 384:512], pg[:, 384:512],
                                         AF.Sigmoid)
                    h_sl = ewp.tile([128, 128], BF16)
                    nc.vector.tensor_mul(h_sl[:], gnl[:, 384:512], tc_t[:])

                # -- exchange: AllGather h_T across the 8 cores --
                if do_store:
                    bnc_in = dramp.tile([128, B], BF16)
                    nc.sync.dma_start(bnc_in[:], h_sl[:])
                if GUT == 5 and t == 0:
                    for kc in range(NKH):
                        nc.sync.dma_start(
                            stat_h[kc * 128:(kc + 1) * 128, :], h_sl[:])
                if do_ag:
                    bnc_out = dramp.tile(
                        [H, B], BF16,
                        addr_space="Local" if NO_CC else "Shared")
                    if NO_CC:
                        for kc in range(NKH):
                            nc.gpsimd.dma_start(
                                bnc_out[kc * 128:(kc + 1) * 128, :], bnc_in[:])
                    else:
                        nc.gpsimd.collective_compute(
                            "AllGather",
                            ALU.bypass,
                            replica_groups=[list(range(N_CORES))],
                            ins=[bnc_in[:].opt()],
                            outs=[bnc_out[:].opt()],
                        )
                elif do_load:
                    bnc_out = stat_h if GUT == 5 else dramp.tile([H, B], BF16)
                if do_load:
                    hT = htp.tile([128, NKH, B], BF16)
                    nc.sync.dma_start(
                        hT[:], bnc_out[:].rearrange("(k p) b -> p k b", p=128))
                    hT_prev = hT

            # head for the final step
            if do_head:
                emit_head(nc, php, pfp, ewp, fc1_sb, fc1b_sb, fc2w_sb,
                          hT_prev, out_acc, t_steps - 1)

            # bulk store of the accumulated head partials (token-major)
            nc.sync.dma_start(
                out_part[:].rearrange("(b t) o -> b t o", t=t_steps),
                out_acc[:])

            # ---- tail: ReduceScatter fc2 partials, bias + log_softmax ----
            rs_out = dramp1.tile([n_tok_loc, O], F32)
            nc.gpsimd.collective_compute(
                "ReduceScatter",
                ALU.add,
                replica_groups=[list(range(N_CORES))],
                ins=[out_part[:].opt()],
                outs=[rs_out[:].opt()],
            )
            # token rows viewed as [128, CH, O]: partition p owns tokens
            # p*CH..(p+1)*CH-1 — contiguous per-partition DMA lines
            CH = n_tok_loc // 128
            z_sb = ewp.tile([128, CH, O], F32)
            nc.sync.dma_start(
                z_sb[:], rs_out[:].rearrange("(p c) o -> p c o", p=128))
            res_sb = ewp.tile([128, CH, O], F32)
            for ch in range(CH):
                zb = ewp.tile([128, O], F32)
                nc.vector.tensor_add(zb[:], z_sb[:, ch, :], fc2b_sb)
                mx = ewp.tile([128, 1], F32)
                nc.vector.reduce_max(mx[:], zb[:], axis=mybir.AxisListType.X)
                sh = ewp.tile([128, O], F32)
                nc.vector.tensor_scalar_sub(sh[:], zb[:], mx[:])
                ex = ewp.tile([128, O], F32)
                nc.scalar.activation(ex[:], sh[:], AF.Exp)
                sm = ewp.tile([128, 1], F32)
                nc.vector.reduce_sum(sm[:], ex[:], axis=mybir.AxisListType.X)
                lg = ewp.tile([128, 1], F32)
                nc.scalar.activation(lg[:], sm[:], AF.Ln)
                nc.vector.tensor_scalar_sub(res_sb[:, ch, :], sh[:], lg[:])
            nc.sync.dma_start(
                out_sh[:].rearrange("(p c) o -> p c o", p=128), res_sb[:])

    nc.compile()
    return nc


def emit_head(nc, php, pfp, ewp, fc1_sb, fc1b_sb, fc2w_sb, hT, out_acc, t):
    """fc1 (C slice) + relu + fc2 partial for timestep t → SBUF accumulator."""
    ph = php.tile([128, B], F32)
    for kc in range(NKH):
        nc.tensor.matmul(ph[:], fc1_sb[:, kc, :], hT[:, kc, :],
                         start=(kc == 0), stop=(kc == NKH - 1))
    hid = ewp.tile([128, B], BF16)
    nc.scalar.activation(hid[:], ph[:], AF.Relu, bias=fc1b_sb)
    pf = pfp.tile([128, O], F32)
    nc.tensor.matmul(pf[:], hid[:], fc2w_sb, start=True, stop=True)
    nc.vector.tensor_copy(out_acc[:, t, :], pf[:])


_CACHED = {}


def _get_kernel(t_steps):
    if t_steps not in _CACHED:
        _CACHED[t_steps] = build_kernel(t_steps)
    return _CACHED[t_steps]


def prep_inputs(m_text, m_audio, m_video, W_ih, W_hh, b_ih, b_hh,
                fc1_w, fc1_b, fc2_w, fc2_b, t_steps=T):
    """Host-side layout prep; returns per-core input maps."""
    bf = ml_dtypes.bfloat16
    x = np.concatenate([np.asarray(m_text), np.asarray(m_audio),
                        np.asarray(m_video)], axis=-1).astype(np.float32)
    b_, t_, d_ = x.shape
    assert (b_, d_) == (B, D) and t_ == t_steps
    # x_T: [T, D+1(ones) padded to 4*128, B], stored [t, p, k, b] so the
    # per-step SBUF load is one contiguous 1KB line per partition
    xTf = np.zeros((t_steps, NKX * 128, B), np.float32)
    xTf[:, :D, :] = x.transpose(1, 2, 0)
    xTf[:, D, :] = 1.0
    xTf = np.ascontiguousarray(
        xTf.reshape(t_steps, NKX, 128, B).transpose(0, 2, 1, 3)).astype(
            ml_dtypes.float8_e4m3)

    W_ih = np.asarray(W_ih, np.float32)
    W_hh = np.asarray(W_hh, np.float32)
    bias = (np.asarray(b_ih) + np.asarray(b_hh)).astype(np.float32)
    fc1_w = np.asarray(fc1_w, np.float32)
    fc1_b = np.asarray(fc1_b, np.float32)
    fc2_w = np.asarray(fc2_w, np.float32)
    fc2_b = np.asarray(fc2_b, np.float32)

    in_maps = []
    gate_order = (0, 1, 2, 3)  # i, f, g, o (pytorch row-block order)
    for k in range(N_CORES):
        rows = np.concatenate(
            [np.arange(g * H + k * HSL, g * H + (k + 1) * HSL)
             for g in gate_order])
        wih_sl = W_ih[rows, :]            # [512, 409]
        whh_sl = W_hh[rows, :]            # [512, 1024]
        b_sl = bias[rows]                 # [512]
        wihT_k = np.zeros((NKX * 128, GSL), np.float32)
        wihT_k[:D, :] = wih_sl.T
        wihT_k[D, :] = b_sl
        # p-major: [128, NKX*GSL]
        wih_pm = wihT_k.reshape(NKX, 128, GSL).transpose(1, 0, 2).reshape(
            128, NKX * GSL)
        whh_pm = whh_sl.T.reshape(NKH, 128, GSL).transpose(1, 0, 2).reshape(
            128, NKH * GSL)

        cc = k % 4                        # C chunk (cores k and k+4 duplicate)
        crows = np.arange(cc * 128, (cc + 1) * 128)
        fc1_pm = fc1_w[crows, :].T.reshape(NKH, 128, 128).transpose(
            1, 0, 2).reshape(128, NKH * 128)
        fc2wT_k = 0.5 * fc2_w[:, crows].T                    # [128, O]
        wblob_k = np.ascontiguousarray(np.concatenate(
            [wih_pm, whh_pm, fc1_pm, fc2wT_k], axis=1)).astype(bf)

        fc1b_k = fc1_b[crows].reshape(128, 1)
        fc2b_k = np.broadcast_to(fc2_b[None, :], (128, O))
        fblob_k = np.ascontiguousarray(np.concatenate(
            [fc1b_k, fc2b_k], axis=1)).astype(np.float32)

        t_sh = t_steps // N_CORES
        in_maps.append({
            "xTs": xTf[k * t_sh:(k + 1) * t_sh],
            "wblob": wblob_k,
            "fblob": fblob_k,
        })
    return in_maps


def run(inputs, t_steps=T, trace=False):
    nc = _get_kernel(t_steps)
    in_maps = prep_inputs(
        inputs["m_text"], inputs["m_audio"], inputs["m_video"],
        inputs["W_ih"], inputs["W_hh"], inputs["b_ih"], inputs["b_hh"],
        inputs["fc1_w"], inputs["fc1_b"], inputs["fc2_w"], inputs["fc2_b"],
        t_steps=t_steps)
    res = run_bass_kernel_spmd(
        nc, in_maps, core_ids=list(range(N_CORES)), trace=trace)
    shards = [res.results[k]["out_sh"] for k in range(N_CORES)]
    full = np.concatenate(shards, axis=0)          # [(b t), O] token-major
    out = full.reshape(B, t_steps, O)
    return out, res


def kernel(**inputs) -> np.ndarray:
    t_steps = np.asarray(inputs["m_text"]).shape[1]
    out, _ = run(inputs, t_steps=t_steps)
    return out.astype(np.float32)

